# revision 1
# baseline (speedup 1.0000x reference)
"""Inverse DTCWT (biort bandpass) level-1 reconstruction as a Bass/Tile kernel.

Math: the reference is
    y = (A0 @ Yl + A1 @ lh) @ A0^T + (A0 @ hl) @ A1^T + (A2 @ hh) @ A2^T
where A* are 256x256 banded matrices (1D taps + symmetric padding folded in)
and lh/hl/hh are the c2q quad-interleaves of subband pairs (0,5)/(2,3)/(1,4).

Row r of a c2q image comes from `top` (r even) or `bot` (r odd), each a
128x256 column-interleaved image built with 3 DVE tensor-tensor ops per pair:
    top[:, 0::2] = w1r + w2r ; top[:, 1::2] = w1i + w2i
    bot[:, 0::2] = w1i - w2i ; bot[:, 1::2] = w2r - w1r
The row interleave never materializes: contraction over rows splits into
even/odd with host-precomputed matrices Re = A^T[0::2]/sqrt2, Ro = A^T[1::2]/sqrt2.

Stage A (col filters) runs with the *image tiles stationary* producing
transposed intermediates Z[c, h] in PSUM; stage B (row filters) consumes Z
slices as stationary against A^T and accumulates all three paths into one
PSUM bank in natural orientation. No transposes anywhere.

All matmuls run in float32r (single-pass fp32 on the TRN2 PE; regular fp32
is a 2-pass mode at half throughput). Inputs/outputs are repacked on the
host into group-major layouts so every DMA moves multi-KB contiguous rows
per partition (the natural layouts would bottleneck at 512B descriptors).

Sharding: pure data parallel, batch dim (8) across 8 cores.
"""
import sys

if "/opt/trn_rl_repo" not in sys.path:
    sys.path.insert(0, "/opt/trn_rl_repo")

import numpy as np

_C, _H = 64, 256  # channels per core, image size
_NCORES = 8
_G = 4  # images (channels) per group


def _band_matrix(h, N):
    """A @ x == colfilter(x, h) with symmetric padding, in float64."""
    h = np.asarray(h, dtype=np.float64)
    L = h.shape[0]
    m = L // 2
    A = np.zeros((N, N), dtype=np.float64)
    for i in range(N):
        for k in range(L):
            s = i + k - m
            if s < 0:
                s = -1 - s
            elif s >= N:
                s = 2 * N - 1 - s
            A[i, s] += h[L - 1 - k]
    return A


def build_consts(g0o, g1o, g2o):
    """Host-side constant tensors handed to every core."""
    A0 = _band_matrix(g0o, _H).T  # stored transposed: [r, h]
    A1 = _band_matrix(g1o, _H).T
    A2 = _band_matrix(g2o, _H).T
    s2 = np.sqrt(2.0)

    def tile2(AT):  # [256, 256] -> [128, 2, 256] with [p, kr, h] = AT[128*kr+p, h]
        return np.ascontiguousarray(
            AT.reshape(2, 128, 256).transpose(1, 0, 2)
        ).astype(np.float32)

    a0t, a1t, a2t = tile2(A0), tile2(A1), tile2(A2)
    # rmats[q, e/o]: per-pair col-filter matrices; pair q uses bands (q, 5-q):
    #   q=0 (lh)   -> col filter A1 ; q=1 (hh) -> A2 ; q=2 (hl) -> A0
    rmats = np.stack(
        [
            np.stack([A1[0::2] / s2, A1[1::2] / s2]),
            np.stack([A2[0::2] / s2, A2[1::2] / s2]),
            np.stack([A0[0::2] / s2, A0[1::2] / s2]),
        ]
    ).astype(np.float32)  # [3, 2, 128, 256]
    return {"a0t": a0t, "a1t": a1t, "a2t": a2t, "rmats": rmats}


def build_nc(n_images):
    import concourse.bacc as bacc
    import concourse.mybir as mybir
    from concourse.tile import TileContext

    f32 = mybir.dt.float32
    f32r = mybir.dt.float32r
    nc = bacc.Bacc(None, target_bir_lowering=False, debug=False)

    n_groups = n_images // _G
    yl_d = nc.declare_dram_parameter(
        "ylp", [n_groups, 128, _G, 2, 256], f32r, isOutput=False
    )
    yh_d = nc.declare_dram_parameter(
        "yhp", [n_groups, 128, _G, 6, 2, 128], f32, isOutput=False
    )
    a0t_d = nc.declare_dram_parameter("a0t", [128, 2, 256], f32r, isOutput=False)
    a1t_d = nc.declare_dram_parameter("a1t", [128, 2, 256], f32r, isOutput=False)
    a2t_d = nc.declare_dram_parameter("a2t", [128, 2, 256], f32r, isOutput=False)
    rm_d = nc.declare_dram_parameter("rmats", [3, 2, 128, 256], f32r, isOutput=False)
    out_d = nc.declare_dram_parameter(
        "out", [n_groups, 128, _G, 2, 256], f32, isOutput=True
    )
    assert n_groups * _G == n_images

    with TileContext(nc) as tc:
        with (
            tc.tile_pool(name="consts", bufs=1) as cpool,
            tc.tile_pool(name="io", bufs=2) as io_pool,
            tc.tile_pool(name="tb", bufs=2) as tb_pool,
            tc.tile_pool(name="zsb", bufs=2) as z_pool,
            tc.tile_pool(name="ps", bufs=2, space="PSUM") as ps_pool,
        ):
            a0t = cpool.tile([128, 2, 256], f32r)
            a1t = cpool.tile([128, 2, 256], f32r)
            a2t = cpool.tile([128, 2, 256], f32r)
            rm = cpool.tile([128, 3, 2, 256], f32r)
            nc.sync.dma_start(rm[:], rm_d[:].rearrange("q e t h -> t q e h"))
            nc.sync.dma_start(a0t[:], a0t_d[:])
            nc.scalar.dma_start(a1t[:], a1t_d[:])
            nc.scalar.dma_start(a2t[:], a2t_d[:])

            for g in range(n_groups):
                yh = io_pool.tile([128, _G, 6, 2, 128], f32, tag="yh", bufs=3)
                yl = io_pool.tile([128, _G, 2, 256], f32r, tag="yl")
                nc.sync.dma_start(yh[:], yh_d[g])
                nc.sync.dma_start(yl[:], yl_d[g])

                top = tb_pool.tile([128, _G, 3, 128, 2], f32r, tag="top", bufs=3)
                bot = tb_pool.tile([128, _G, 3, 128, 2], f32r, tag="bot", bufs=3)
                for q in range(3):
                    # all images at once; w1 = band q, w2 = band 5-q
                    w1 = yh[:, :, q, :, :].transpose([0, 1, 3, 2])  # (p, i, w, ri)
                    w2 = yh[:, :, 5 - q, :, :].transpose([0, 1, 3, 2])
                    nc.vector.tensor_add(top[:, :, q, :, :], w1, w2)
                    nc.vector.tensor_sub(
                        bot[:, :, q, :, 0], yh[:, :, q, 1, :], yh[:, :, 5 - q, 1, :]
                    )
                    nc.vector.tensor_sub(
                        bot[:, :, q, :, 1], yh[:, :, 5 - q, 0, :], yh[:, :, q, 0, :]
                    )

                out_sb = io_pool.tile([128, _G, 2, 256], f32, tag="out_sb")
                for i in range(_G):
                    # ---- stage A: Z[c, h] = col-filtered, transposed ----
                    z1 = ps_pool.tile([128, 2, 256], f32, tag="z1")
                    z2 = ps_pool.tile([128, 2, 256], f32, tag="z2")
                    z3 = ps_pool.tile([128, 2, 256], f32, tag="z3")
                    for cc in range(2):
                        js = slice(64 * cc, 64 * cc + 64)
                        ws = slice(128 * cc, 128 * cc + 128)
                        # z1: lh path (pair q=0, col A1) + Yl path (col A0)
                        nc.tensor.matmul(
                            z1[:, cc, :], top[:, i, 0, js, :], rm[:, 0, 0, :],
                            start=True, stop=False,
                        )
                        nc.tensor.matmul(
                            z1[:, cc, :], bot[:, i, 0, js, :], rm[:, 0, 1, :],
                            start=False, stop=False,
                        )
                        nc.tensor.matmul(
                            z1[:, cc, :], yl[:, i, 0, ws], a0t[:, 0, :],
                            start=False, stop=False,
                        )
                        nc.tensor.matmul(
                            z1[:, cc, :], yl[:, i, 1, ws], a0t[:, 1, :],
                            start=False, stop=True,
                        )
                        # z2: hl path (pair q=2, col A0); row filter A1 later
                        nc.tensor.matmul(
                            z2[:, cc, :], top[:, i, 2, js, :], rm[:, 2, 0, :],
                            start=True, stop=False,
                        )
                        nc.tensor.matmul(
                            z2[:, cc, :], bot[:, i, 2, js, :], rm[:, 2, 1, :],
                            start=False, stop=True,
                        )
                        # z3: hh path (pair q=1, col A2); row filter A2 later
                        nc.tensor.matmul(
                            z3[:, cc, :], top[:, i, 1, js, :], rm[:, 1, 0, :],
                            start=True, stop=False,
                        )
                        nc.tensor.matmul(
                            z3[:, cc, :], bot[:, i, 1, js, :], rm[:, 1, 1, :],
                            start=False, stop=True,
                        )
                    z1s = z_pool.tile([128, 2, 256], f32r, tag="z1s")
                    z2s = z_pool.tile([128, 2, 256], f32r, tag="z2s")
                    z3s = z_pool.tile([128, 2, 256], f32r, tag="z3s")
                    nc.scalar.copy(z1s[:], z1[:])
                    nc.scalar.copy(z2s[:], z2[:])
                    nc.scalar.copy(z3s[:], z3[:])

                    # ---- stage B: y[r, cout] = sum_paths Z^T @ A^T ----
                    yp = ps_pool.tile([128, 2, 256], f32, tag="yp")
                    for r in range(2):
                        rs = slice(128 * r, 128 * r + 128)
                        for pi, (zs, amat) in enumerate(
                            ((z1s, a0t), (z2s, a1t), (z3s, a2t))
                        ):
                            nc.tensor.matmul(
                                yp[:, r, :], zs[:, 0, rs], amat[:, 0, :],
                                start=(pi == 0), stop=False,
                            )
                            nc.tensor.matmul(
                                yp[:, r, :], zs[:, 1, rs], amat[:, 1, :],
                                start=False, stop=(pi == 2),
                            )
                    nc.scalar.copy(out_sb[:, i, :, :], yp[:])

                nc.scalar.dma_start(out_d[g], out_sb[:])
    nc.compile()
    return nc


_NC_CACHE = {}


def _get_nc(n_images):
    if n_images not in _NC_CACHE:
        _NC_CACHE[n_images] = build_nc(n_images)
    return _NC_CACHE[n_images]


def pack_inputs(Yl_k, Yhr_k, Yhi_k):
    """Per-core repack into group-major layouts with long contiguous rows.

    yhp[g, h, i, s, ri, w] = (Yhr|Yhi)[4g+i, s, h, w] -> 24KB/partition/group
    ylp[g, p, i, k, w] = Yl[4g+i, 128k+p, w]          ->  8KB/partition/group
    """
    ng = _C // _G
    yhp = np.empty((ng, 128, _G, 6, 2, 128), dtype=np.float32)
    yhp[:, :, :, :, 0, :] = (
        Yhr_k.reshape(ng, _G, 6, 128, 128).transpose(0, 3, 1, 2, 4)
    )
    yhp[:, :, :, :, 1, :] = (
        Yhi_k.reshape(ng, _G, 6, 128, 128).transpose(0, 3, 1, 2, 4)
    )
    ylp = np.ascontiguousarray(
        Yl_k.reshape(ng, _G, 2, 128, 256).transpose(0, 3, 1, 2, 4)
    )
    return yhp, ylp


def unpack_output(outp):
    """outp (ng, 128, G, 2, 256): [g, p, i, k, w] = y[Gg+i, 128k+p, w]."""
    return np.ascontiguousarray(
        outp.transpose(0, 2, 3, 1, 4).reshape(outp.shape[0] * _G, 256, 256)
    )


def kernel(Yl, Yhr, Yhi, g0o, g1o, g2o):
    from concourse.bass_utils import run_bass_kernel_spmd

    Yl = np.asarray(Yl, dtype=np.float32)
    Yhr = np.asarray(Yhr, dtype=np.float32)
    Yhi = np.asarray(Yhi, dtype=np.float32)
    consts = build_consts(np.asarray(g0o), np.asarray(g1o), np.asarray(g2o))

    nc = _get_nc(_C)
    in_maps = []
    for k in range(_NCORES):
        yhp, ylp = pack_inputs(Yl[k], Yhr[k], Yhi[k])
        in_maps.append({"ylp": ylp, "yhp": yhp, **consts})
    res = run_bass_kernel_spmd(nc, in_maps, list(range(_NCORES)))
    out = np.stack([unpack_output(res.results[k]["out"]) for k in range(_NCORES)])
    return out.astype(np.float32)



# revision 4
# speedup vs baseline: 1.5385x; 1.5385x over previous
"""Inverse DTCWT (biort bandpass) level-1 reconstruction as a Bass/Tile kernel.

Math: the reference is
    y = M0c @ Yl @ M0r' + M1c @ LH @ M0r' + M0c @ HL @ M1r' + M2c @ HH @ M2r'
where M* are 256x256 banded matrices (1D taps + symmetric padding folded in)
and LH/HL/HH are the c2q quad-interleaves of subband pairs (0,5)/(2,3)/(1,4).

All c2q sums/differences and every layout shuffle run on the HOST (numpy);
the device sees three bf16 streams:
  tb:  per pair, [top rows; bot rows] halves stacked across partitions so a
       single 128-contraction matmul applies both the even-row and odd-row
       column-filter taps in one pass,
  yl:  the lowpass image split in two 128-row chunks,
  flt: all banded filter matrices pre-sliced to their nonzero column extents.
Because the filters are banded (13/19 taps -> halfwidth 6/9), each matmul's
moving extent is ~134-138 columns instead of 256 - nearly halving PE streaming
time vs dense 256-wide passes. Region-split accumulation into one PSUM bank is
legal because start=True clears has_written for the whole bank and start=False
matmuls overwrite-where-clear / accumulate-where-set per element.

Everything is bf16 (inputs, weights, z intermediates, output; PSUM stays
fp32): halves DMA traffic vs fp32 and enables Fast Weight Load (2x faster
LDWEIGHTS than fp32). rel-err vs fp32 reference ~3e-3.

Sharding: pure data parallel, batch dim (8) across 8 cores.
"""
import sys

if "/opt/trn_rl_repo" not in sys.path:
    sys.path.insert(0, "/opt/trn_rl_repo")

import numpy as np
import ml_dtypes

BF16 = ml_dtypes.bfloat16

_C, _H = 64, 256  # channels per core, image size
_NCORES = 8
_G = 4            # images (channels) per group
_NG = _C // _G    # 16 groups

# pair q -> (band1, band2, col-filter id); filter ids: 0=g0o(13) 1=g1o(19) 2=g2o(13)
_PAIRS = [(0, 5, 1), (1, 4, 2), (2, 3, 0)]
# stage B: (z index, row-filter id) in emission order
_ROWMAP = [(0, 0), (2, 1), (1, 2)]
_HALF = {0: 6, 1: 9, 2: 6}  # filter halfwidths (L//2)


def _ext(m):
    """Even-aligned (lo, hi) output-column extents for a halfwidth-m band
    matrix split at row 128: lo rows 0:128 touch cols [0, 128+m),
    hi rows 128:256 touch cols [128-m, 256)."""
    lo_end = 128 + m
    lo_end += lo_end % 2
    hi_start = 128 - m
    hi_start -= hi_start % 2
    return (0, lo_end), (hi_start, 256)


def _flt_layout():
    """Static layout of the packed filter tensor [128, T]:
    entries keyed (kind, idx, half) -> (offset, h0, width)."""
    lay, off = {}, 0
    def add(key, m):
        nonlocal off
        (l0, l1), (h0, h1) = _ext(m)
        for half, (a, b) in ((0, (l0, l1)), (1, (h0, h1))):
            lay[key + (half,)] = (off, a, b - a)
            off += b - a
    for q, (_, _, f) in enumerate(_PAIRS):
        add(("A", q), _HALF[f])
    add(("YL", 0), _HALF[0])
    for p, (_, f) in enumerate(_ROWMAP):
        add(("B", p), _HALF[f])
    return lay, off


_FLT_LAY, _FLT_T = _flt_layout()


def _band_matrix(h, N):
    """M @ x == colfilter(x, h) with symmetric padding, in float64."""
    h = np.asarray(h, dtype=np.float64)
    L = h.shape[0]
    m = L // 2
    A = np.zeros((N, N), dtype=np.float64)
    for i in range(N):
        for k in range(L):
            s = i + k - m
            if s < 0:
                s = -1 - s
            elif s >= N:
                s = 2 * N - 1 - s
            A[i, s] += h[L - 1 - k]
    return A


def build_consts(g0o, g1o, g2o):
    """Pack every filter block into one [128, T] bf16 tensor."""
    Ms = [_band_matrix(g, _H) for g in (g0o, g1o, g2o)]
    s2 = np.sqrt(2.0)
    flt = np.zeros((128, _FLT_T), dtype=np.float64)

    def put(key, block):
        off, h0, w = _FLT_LAY[key]
        assert block.shape == (128, w), (key, block.shape, w)
        flt[:, off:off + w] = block

    for q, (_, _, f) in enumerate(_PAIRS):
        MT = Ms[f].T  # [src_row, out_col]
        ReT, RoT = MT[0::2] / s2, MT[1::2] / s2  # [128, 256]
        for half in (0, 1):
            off, h0, w = _FLT_LAY[("A", q, half)]
            sl = slice(64 * half, 64 * half + 64)
            put(("A", q, half), np.vstack([ReT[sl], RoT[sl]])[:, h0:h0 + w])
    M0T = Ms[0].T
    for half in (0, 1):
        off, h0, w = _FLT_LAY[("YL", 0, half)]
        put(("YL", 0, half), M0T[128 * half:128 * half + 128, h0:h0 + w])
    for p, (_, f) in enumerate(_ROWMAP):
        MT = Ms[f].T
        for half in (0, 1):
            off, h0, w = _FLT_LAY[("B", p, half)]
            put(("B", p, half), MT[128 * half:128 * half + 128, h0:h0 + w])
    return {"flt": flt.astype(BF16)}


def build_nc(n_images):
    import concourse.bacc as bacc
    import concourse.mybir as mybir
    from concourse.tile import TileContext

    f32 = mybir.dt.float32
    bf16 = mybir.dt.bfloat16
    nc = bacc.Bacc(None, target_bir_lowering=False, debug=False)

    ng = n_images // _G
    assert ng * _G == n_images
    tb_d = nc.declare_dram_parameter(
        "tbp", [ng, 128, _G, 3, 2, 256], bf16, isOutput=False
    )
    yl_d = nc.declare_dram_parameter(
        "ylp", [ng, 128, _G, 2, 256], bf16, isOutput=False
    )
    flt_d = nc.declare_dram_parameter("flt", [128, _FLT_T], bf16, isOutput=False)
    out_d = nc.declare_dram_parameter(
        "out", [ng, 128, _G, 2, 256], bf16, isOutput=True
    )

    def fslice(flt_sb, key):
        off, h0, w = _FLT_LAY[key]
        return flt_sb[:, off:off + w], h0, w

    with TileContext(nc) as tc:
        with (
            tc.tile_pool(name="consts", bufs=1) as cpool,
            tc.tile_pool(name="io", bufs=2) as io_pool,
            tc.tile_pool(name="zsb", bufs=2) as z_pool,
            tc.tile_pool(name="ps", bufs=2, space="PSUM") as ps_pool,
        ):
            flt = cpool.tile([128, _FLT_T], bf16)
            nc.sync.dma_start(flt[:], flt_d[:])

            for g in range(ng):
                tb = io_pool.tile([128, _G, 3, 2, 256], bf16, tag="tb", bufs=3)
                yl = io_pool.tile([128, _G, 2, 256], bf16, tag="yl", bufs=3)
                nc.sync.dma_start(tb[:], tb_d[g])
                nc.scalar.dma_start(yl[:], yl_d[g])

                out_sb = io_pool.tile([128, _G, 2, 256], bf16, tag="out_sb")
                for i in range(_G):
                    # ---- stage A: z[q][col, h], col filters folded in ----
                    z = [
                        ps_pool.tile(
                            [128, 2, 256], f32, tag=f"z{q}", name=f"z{q}"
                        )
                        for q in range(3)
                    ]
                    for q in range(3):
                        for cc in range(2):
                            cs = slice(128 * cc, 128 * cc + 128)
                            for half in (0, 1):
                                mv, h0, w = fslice(flt, ("A", q, half))
                                nc.tensor.matmul(
                                    z[q][:, cc, h0:h0 + w],
                                    tb[:, i, q, half, cs],
                                    mv,
                                    start=(cc == 0 and half == 0),
                                    stop=(q != 0 and cc == 1 and half == 1),
                                )
                    # lowpass path accumulates into z[0]
                    for cc in range(2):
                        cs = slice(128 * cc, 128 * cc + 128)
                        for k in range(2):
                            mv, h0, w = fslice(flt, ("YL", 0, k))
                            nc.tensor.matmul(
                                z[0][:, cc, h0:h0 + w],
                                yl[:, i, k, cs],
                                mv,
                                start=False,
                                stop=(cc == 1 and k == 1),
                            )

                    zsb = z_pool.tile([128, 3, 2, 256], bf16, tag="zsb")
                    nc.vector.tensor_copy(out=zsb[:, 0], in_=z[0][:])
                    nc.vector.tensor_copy(out=zsb[:, 1], in_=z[1][:])
                    nc.scalar.copy(zsb[:, 2], z[2][:])

                    # ---- stage B: y[h, w] = sum_paths z^T @ row-filter ----
                    yp = ps_pool.tile([128, 2, 256], f32, tag="yp")
                    for r in range(2):
                        rs = slice(128 * r, 128 * r + 128)
                        for p, (zi, _) in enumerate(_ROWMAP):
                            for cc in range(2):
                                mv, h0, w = fslice(flt, ("B", p, cc))
                                nc.tensor.matmul(
                                    yp[:, r, h0:h0 + w],
                                    zsb[:, zi, cc, rs],
                                    mv,
                                    start=(r == 0 and p == 0 and cc == 0),
                                    stop=(r == 1 and p == 2 and cc == 1),
                                )
                    nc.scalar.copy(out_sb[:, i, :, :], yp[:])

                nc.scalar.dma_start(out_d[g], out_sb[:])
    nc.compile()
    return nc


_NC_CACHE = {}


def _get_nc(n_images):
    if n_images not in _NC_CACHE:
        _NC_CACHE[n_images] = build_nc(n_images)
    return _NC_CACHE[n_images]


def pack_inputs(Yl_k, Yhr_k, Yhi_k):
    """Per-core repack (c2q on host) into bf16 group-major layouts.

    tbp[g, p, i, q, s, c]: pair-q c2q data for channel 4g+i; partitions hold
      [top rows 64s:64s+64 ; bot rows 64s:64s+64] stacked; c = 2w + (r/i).
    ylp[g, p, i, k, w] = Yl[4g+i, 128k+p, w]
    """
    C = Yl_k.shape[0]
    ng = C // _G
    tbp = np.empty((ng, 128, _G, 3, 2, 256), dtype=BF16)
    for q, (b1, b2, _) in enumerate(_PAIRS):
        w1r, w1i = Yhr_k[:, b1], Yhi_k[:, b1]   # [C, 128, 128]
        w2r, w2i = Yhr_k[:, b2], Yhi_k[:, b2]
        top = np.empty((C, 128, 256), dtype=np.float32)
        bot = np.empty((C, 128, 256), dtype=np.float32)
        top[:, :, 0::2] = w1r + w2r
        top[:, :, 1::2] = w1i + w2i
        bot[:, :, 0::2] = w1i - w2i
        bot[:, :, 1::2] = w2r - w1r
        for s in range(2):
            hs = slice(64 * s, 64 * s + 64)
            # [C, 128, 256] -> [ng, G, 128, 256] -> [ng, 128, G, 256]
            stk = np.concatenate([top[:, hs], bot[:, hs]], axis=1)
            tbp[:, :, :, q, s, :] = (
                stk.reshape(ng, _G, 128, 256).transpose(0, 2, 1, 3)
            )
    ylp = np.ascontiguousarray(
        Yl_k.reshape(ng, _G, 2, 128, 256).transpose(0, 3, 1, 2, 4)
    ).astype(BF16)
    return tbp, ylp


def unpack_output(outp):
    """outp (ng, 128, G, 2, 256): [g, p, i, k, w] = y[G*g+i, 128k+p, w]."""
    return np.ascontiguousarray(
        outp.astype(np.float32).transpose(0, 2, 3, 1, 4).reshape(-1, 256, 256)
    )


def kernel(Yl, Yhr, Yhi, g0o, g1o, g2o):
    from concourse.bass_utils import run_bass_kernel_spmd

    Yl = np.asarray(Yl, dtype=np.float32)
    Yhr = np.asarray(Yhr, dtype=np.float32)
    Yhi = np.asarray(Yhi, dtype=np.float32)
    consts = build_consts(np.asarray(g0o), np.asarray(g1o), np.asarray(g2o))

    nc = _get_nc(_C)
    in_maps = []
    for k in range(_NCORES):
        tbp, ylp = pack_inputs(Yl[k], Yhr[k], Yhi[k])
        in_maps.append({"ylp": ylp, "tbp": tbp, **consts})
    res = run_bass_kernel_spmd(nc, in_maps, list(range(_NCORES)))
    out = np.stack([unpack_output(res.results[k]["out"]) for k in range(_NCORES)])
    return out.astype(np.float32)
